# revision 35
# baseline (speedup 1.0000x reference)
"""Trainium2 Bass kernel: transformer block (QKV proj + MHA + residual + LN +
MLP(relu) residual + LN) for B=2, S=4096, D=512, H=8.

Sharding: data-parallel over (batch, query-row-block) — 8 cores x 1024 query
rows. Each core recomputes K/V projections for its batch (4 cores share a
batch), attends over all 4096 keys, and runs the per-row tail. No cross-core
communication.

Layouts: feature-major ("T" = [d, rows]) so projection/attention matmuls chain
without re-transposing. Softmax sums come free from a ones-column appended to
each V tile. fp32r matmuls (full PE rate at moving-dim >= 256).
"""

import math

import numpy as np

import concourse.bass as bass
import concourse.mybir as mybir
import concourse.tile as tile
from concourse.masks import make_identity

# ---------------------------------------------------------------------------
# Workaround: this walrus build rejects >1 sync-wait on the TileContext exit
# drain (CoreV3 setupSyncWait "Too many sync wait commands"). Split the waits
# across single-wait NOPs.
_orig_drain_and_barrier = tile.TileContext._drain_and_barrier


def _split_drain_and_barrier(self, tick_clock, wait_clock):
    from concourse.tile import ScopedClock

    nc = self.nc
    drain_inst = nc.sync.drain()
    wait_clock.add_sem_waits(
        drain_inst.ins, ScopedClock({None: tick_clock.global_clock})
    )
    si = drain_inst.ins.sync_info
    waits = list(si.on_wait) if si and si.on_wait else []
    if len(waits) > 1:
        si.on_wait = waits[:1]
        for w in waits[1:]:
            nop = nc.sync.nop(nofuse=True, hint="drain_wait_split")
            nop.ins.sync_info = mybir.SyncInfo(on_wait=[w], on_update=[])
    nc.all_engine_barrier()
    assert self.sems is not None
    popped = nc._tile_sem_poison_stack.pop()
    assert popped is self._sem_poison
    nc.clear_and_free_semaphores(list(self.sems.allocated().values()))
    nc.all_engine_barrier()


tile.TileContext._drain_and_barrier = _split_drain_and_barrier


def _split_multi_waits(nc, limit=1):
    """walrus CoreV3 codegen caps sync-waits per instruction descriptor; hoist
    excess waits onto fresh NOPs inserted just before the instruction on the
    same engine queue. Matmul (S3_LW) and Drain (CTRL_NO) descriptors only
    fit 1 wait; other engine descriptors fit 2."""
    ctr = [0]

    def mknop(engine, wait):
        ctr[0] += 1
        nop = mybir.InstNoOp(name=f"WSPLIT-{ctr[0]}", ins=[], outs=[])
        nop.engine = engine
        nop.sync_info = mybir.SyncInfo(on_wait=[wait], on_update=[])
        return nop

    nsplit = 0
    for f in nc.m.functions:
        for bb in f.blocks:
            insts = bb.instructions
            i = 0
            while i < len(insts):
                ins = insts[i]
                si = getattr(ins, "sync_info", None)
                if si is not None and si.on_wait and len(si.on_wait) > limit:
                    waits = list(si.on_wait)
                    si.on_wait = waits[-limit:]
                    pre = [mknop(ins.engine, w) for w in waits[:-limit]]
                    for j, p in enumerate(pre):
                        insts.insert(i + j, p)
                        nc.register_instruction(p, overwrite=True)
                    i += len(pre)
                    nsplit += 1
                i += 1
    return nsplit, ctr[0]
# ---------------------------------------------------------------------------

B, S, D, H, DH = 2, 4096, 512, 8, 64
P = 128
NC = 8          # cores
RPC = 1024      # query rows per core
NKC = D // P    # 4 contraction chunks of 128
EPS = 1e-5
SCALE = 1.0 / math.sqrt(D)

F32 = mybir.dt.float32
F32R = mybir.dt.float32r
BF16 = mybir.dt.bfloat16
FP8 = mybir.dt.float8e4
I32 = mybir.dt.int32
U8 = mybir.dt.uint8
ALU = mybir.AluOpType
AF = mybir.ActivationFunctionType

# Schraudolph exp for fp8e4 bit patterns: bits = round(8*log2(exp(s*SCALE)))+56
# = s * (8*log2(e)*SCALE) + 56 (+ rounding-bias correction). Max rel err ~7%
# in the normal range; only the (negligible-weight) denormal tail is worse.
A_EXP = 8.0 * math.log2(math.e) * SCALE
B_EXP = 55.55
# every DVE_MOD-th c-block's exp runs on DVE instead of ACT
DVE_MOD = 2


def r(ap):
    """view an fp32 AP as float32r for full-rate PE matmuls"""
    return ap.bitcast(F32R)


def build_nc(reps=1, trivial=True):
    """trivial=True assumes zero q/k/v/o biases and identity layernorm
    params (g=1, b=0) — checked at runtime in kernel(), which falls back
    to the general variant otherwise."""
    nc = bass.Bass()

    # host feeds feature-major (pre-transposed) Q and K slices
    QT = nc.dram_tensor("QT", [D, RPC], F32R, kind="ExternalInput")
    KTb = nc.dram_tensor("KTb", [D, S], F32R, kind="ExternalInput")
    Wq = nc.dram_tensor("Wq", [D, D], F32R, kind="ExternalInput")
    Wk = nc.dram_tensor("Wk", [D, D], F32R, kind="ExternalInput")
    Wv = nc.dram_tensor("Wv", [D, D], F32R, kind="ExternalInput")
    Wo = nc.dram_tensor("Wo", [D, D], F32R, kind="ExternalInput")
    bq = nc.dram_tensor("bq", [D], F32, kind="ExternalInput")
    bk = nc.dram_tensor("bk", [D], F32, kind="ExternalInput")
    bv = nc.dram_tensor("bv", [D], F32, kind="ExternalInput")
    bo = nc.dram_tensor("bo", [D], F32, kind="ExternalInput")
    g0 = nc.dram_tensor("g0", [D], F32, kind="ExternalInput")
    b0 = nc.dram_tensor("b0", [D], F32, kind="ExternalInput")
    g1 = nc.dram_tensor("g1", [D], F32, kind="ExternalInput")
    b1 = nc.dram_tensor("b1", [D], F32, kind="ExternalInput")
    Oo = nc.dram_tensor("O", [RPC, D], F32, kind="ExternalOutput")

    def bcast_ap(dram_vec):
        # [cols] dram vector -> [P, cols] partition-broadcast AP
        a = dram_vec[:]
        return bass.AP(
            tensor=a.tensor,
            offset=a.offset,
            ap=[[0, P]] + list(a.ap),
        )

    def chunked_ap(dram_vec):
        # [D] dram vector -> [P, NKC]: partition = idx within 128-chunk
        return dram_vec.rearrange("(c p) -> p c", p=P)

    with tile.TileContext(nc) as tc:
        with (
            tc.tile_pool(name="consts", bufs=1) as consts,
            tc.tile_pool(name="kT_p", bufs=1) as kT_p,
            tc.tile_pool(name="vx_p", bufs=1) as vx_p,
            tc.tile_pool(name="qT_p", bufs=1) as qT_p,
            tc.tile_pool(name="w_p", bufs=3) as w_p,
            tc.tile_pool(name="ktc_p", bufs=2) as ktc_p,
            tc.tile_pool(name="kload_p", bufs=2) as kload_p,
            tc.tile_pool(name="at_p", bufs=7) as at_p,
            tc.tile_pool(name="ot_p", bufs=2) as ot_p,
            tc.tile_pool(name="opre_p", bufs=7) as opre_p,
            tc.tile_pool(name="tail_p", bufs=2) as tail_p,
            tc.tile_pool(name="stat_p", bufs=4) as stat_p,
            tc.tile_pool(name="ps_sc", bufs=3, space="PSUM") as ps_sc,
            tc.tile_pool(name="ps_o", bufs=1, space="PSUM") as ps_o,
        ):
          for _rep in range(reps):
            # ---- constants ----
            ident = consts.tile([P, P], F32)
            make_identity(nc, ident)

            SC2 = 512
            RB = 512

            # weights + first QTc/KTc loaded in interleaved 256KB chunks so
            # the first projection matmuls start as early as possible
            wq_t = w_p.tile([P, NKC, D], F32R, tag="w", name="wq_t")
            wk_t = w_p.tile([P, NKC, D], F32R, tag="w", name="wk_t")
            QTc0 = ktc_p.tile([P, NKC, SC2], F32R, tag="ktc", name="QTc0")
            KTc0 = ktc_p.tile([P, NKC, SC2], F32R, tag="ktc", name="KTc0")
            wq_d = Wq.rearrange("(c p) n -> p c n", p=P)
            wk_d = Wk.rearrange("(c p) n -> p c n", p=P)
            qt_d = QT.rearrange("(c p) r -> p c r", p=P)
            kt_d = KTb.rearrange("(c p) r -> p c r", p=P)
            for kc in range(NKC):
                nc.sync.dma_start(wq_t[:, kc], wq_d[:, kc])
                nc.sync.dma_start(QTc0[:, kc], qt_d[:, kc, 0:SC2])
            for kc in range(NKC):
                nc.gpsimd.dma_start(wk_t[:, kc], wk_d[:, kc])
                nc.gpsimd.dma_start(KTc0[:, kc], kt_d[:, kc, 0:SC2])
            bias_sb = consts.tile([P, 3, NKC], F32)
            if not trivial:
                for wi, bvec in enumerate((bq, bk, bv)):
                    nc.gpsimd.dma_start(bias_sb[:, wi], chunked_ap(bvec))
            bvb = consts.tile([P, H, DH], F32)
            bob = consts.tile([P, D], F32)
            g0b = consts.tile([P, D], F32)
            b0b = consts.tile([P, D], F32)
            g1b = consts.tile([P, D], F32)
            b1b = consts.tile([P, D], F32)
            if not trivial:
                nc.gpsimd.dma_start(
                    bvb, bcast_ap(bv).rearrange("p (h d) -> p h d", h=H))
                for t, v in ((bob, bo), (g0b, g0), (b0b, b0),
                             (g1b, g1), (b1b, b1)):
                    nc.gpsimd.dma_start(t, bcast_ap(v))

            # ---- persistent activations ----
            kT = kT_p.tile([P, NKC, S], F32R)          # (K Wk + bk)^T
            # V rows (fp8) + ones col at DH + zero pad at DH+1 so the
            # DoubleRow Ko stride (H*(DH+2) = 528 B) is 16-byte aligned
            v_ext = vx_p.tile([P, S // P, H, DH + 2], FP8)
            qT = qT_p.tile([P, NKC, RPC], F32R)        # (Q Wq + bq)^T
            nc.vector.memset(v_ext[:, :, :, DH:DH + 1], 1.0)
            nc.vector.memset(v_ext[:, :, :, DH + 1:DH + 2], 0.0)

            def psum_drain(dst, pp, bias_ap):
                """psum -> SBUF with optional per-partition bias add"""
                if trivial:
                    nc.vector.tensor_copy(dst, pp)
                else:
                    nc.vector.tensor_scalar_add(dst, pp, bias_ap)

            # ---- phase B: qT = Wq^T Q^T + bq (Q^T DMA'd directly) ----
            def b_step(rc2, pre=None):
                if pre is None:
                    QTc = ktc_p.tile([P, NKC, SC2], F32R, tag="ktc", name="QTc")
                    nc.sync.dma_start(QTc, qt_d[:, :, rc2 * SC2:(rc2 + 1) * SC2])
                else:
                    QTc = pre
                for ci in range(NKC):
                    pp = ps_sc.tile([P, SC2], F32, tag="psc", name="ppq")
                    for kc in range(NKC):
                        nc.tensor.matmul(
                            pp,
                            lhsT=wq_t[:, kc, ci * P:(ci + 1) * P],
                            rhs=QTc[:, kc],
                            start=(kc == 0), stop=(kc == NKC - 1),
                        )
                    psum_drain(qT[:, ci, rc2 * SC2:(rc2 + 1) * SC2], pp,
                               bias_sb[:, 0, ci:ci + 1])

            b_step(0, pre=QTc0)

            # ---- helpers ----
            def a_step(sc2, pre=None):
                """project K cols [sc2*512, (sc2+1)*512) into kT and v_ext"""
                if pre is None:
                    KTc = ktc_p.tile([P, NKC, SC2], F32R, tag="ktc", name="KTc")
                    nc.sync.dma_start(KTc, kt_d[:, :, sc2 * SC2:(sc2 + 1) * SC2])
                else:
                    KTc = pre
                for ci in range(NKC):
                    pp = ps_sc.tile([P, SC2], F32, tag="psc", name="ppk")
                    for kc in range(NKC):
                        nc.tensor.matmul(
                            pp,
                            lhsT=wk_t[:, kc, ci * P:(ci + 1) * P],
                            rhs=KTc[:, kc],
                            start=(kc == 0), stop=(kc == NKC - 1),
                        )
                    psum_drain(kT[:, ci, sc2 * SC2:(sc2 + 1) * SC2], pp,
                               bias_sb[:, 1, ci:ci + 1])
                for half in range(SC2 // P):
                    pv = ps_sc.tile([P, D], F32, tag="psc", name="ppv")
                    for kc in range(NKC):
                        nc.tensor.matmul(
                            pv,
                            lhsT=KTc[:, kc, half * P:(half + 1) * P],
                            rhs=wv_t[:, kc],
                            start=(kc == 0), stop=(kc == NKC - 1),
                        )
                    sidx = sc2 * (SC2 // P) + half
                    pvh = pv.rearrange("p (h d) -> p h d", h=H)
                    if trivial:
                        nc.vector.tensor_copy(v_ext[:, sidx, :, 0:DH], pvh)
                    else:
                        nc.vector.scalar_tensor_tensor(
                            out=v_ext[:, sidx, :, 0:DH],
                            in0=pvh, scalar=1.0, in1=bvb,
                            op0=ALU.mult, op1=ALU.add,
                        )

            NSP = S // P // 2  # 16 key scpairs (256 keys each)
            PV_LAG = 5         # PV matmuls trail scores/exp by this many blocks

            def c_scores(rb, pair, i, scpair):
                """scores+exp for head 2*pair+i over keys
                [scpair*256, scpair*256+256) x queries [rb*RB, rb*RB+RB).
                Returns the fp8 exp tile for the trailing PV matmul."""
                h = 2 * pair + i
                ci, po = h // 2, (h % 2) * DH
                ps = ps_sc.tile([P, 2, RB], F32, tag="psc", name="psc")
                for j in (0, 1):
                    sc = 2 * scpair + j
                    nc.tensor.matmul(
                        ps[:, j],
                        lhsT=kT[po:po + DH, ci, sc * P:(sc + 1) * P],
                        rhs=qT[po:po + DH, ci, rb * RB:(rb + 1) * RB],
                        start=True, stop=True,
                    )
                at = at_p.tile([P, 2, RB], FP8, tag="at", name="at")
                if (scpair * 2 + i) % DVE_MOD == DVE_MOD - 1:
                    nc.vector.tensor_scalar(
                        at.bitcast(U8), ps, A_EXP, B_EXP, ALU.mult, ALU.add
                    )
                else:
                    nc.scalar.activation(at, ps, AF.Exp, scale=SCALE)
                return (at, i, scpair, h)

            def c_pv(blk):
                at, i, scpair, h, obig, post = blk
                nc.tensor.matmul(
                    obig[0:DH + 1, i],
                    lhsT=v_ext[:, 2 * scpair:2 * scpair + 2, h, 0:DH + 1],
                    rhs=at,
                    start=(scpair == 0), stop=(scpair == NSP - 1),
                    perf_mode=mybir.MatmulPerfMode.DoubleRow,
                )
                if post is not None:
                    post()

            deferred_post = []

            def c_post(rb, pair, obig, opre):
                # copy obig out of PSUM now (frees the accumulator slot);
                # defer the transpose+normalize (which needs psum slots) so
                # it doesn't contend with the next pair's score tiles
                for i, h in enumerate((2 * pair, 2 * pair + 1)):
                    ot = ot_p.tile([DH + 1, RB], F32, tag="ot", name="ot")
                    nc.vector.tensor_copy(ot, obig[0:DH + 1, i])
                    deferred_post.append((ot, h, opre))

            def c_post_finish():
                while deferred_post:
                    ot, h, opre = deferred_post.pop(0)
                    for rc in range(RB // P):
                        pt = ps_sc.tile([P, P], F32, tag="psc", name="pto")
                        nc.tensor.transpose(
                            pt[:, 0:DH + 1], ot[:, rc * P:(rc + 1) * P],
                            ident[0:DH + 1, 0:DH + 1]
                        )
                        rec = stat_p.tile([P, 1], F32, tag="rec", name="rec")
                        nc.vector.reciprocal(rec, pt[:, DH:DH + 1])
                        nc.vector.tensor_scalar_mul(
                            opre[rc][:, h], pt[:, 0:DH], rec
                        )

            def layernorm(dst, src, gb, bb, on_act=False):
                st6 = stat_p.tile([P, 6], F32, tag="st6", name="st6")
                nc.vector.bn_stats(st6, src)
                mv = stat_p.tile([P, 2], F32, tag="mv", name="mv")
                nc.vector.bn_aggr(mv, st6)
                # rstd = rsqrt(var+eps) on DVE: quake seed + 1 Newton step
                # (keeps ACT's table on Exp during the attention stream)
                srt = stat_p.tile([P, 1], F32, tag="srt", name="srt")
                nc.vector.tensor_scalar_add(srt, mv[:, 1:2], EPS)
                yv = stat_p.tile([P, 1], F32, tag="rstd", name="yv")
                yu = yv.bitcast(I32)
                nc.vector.tensor_scalar(yu, srt.bitcast(I32), 1, None,
                                        ALU.logical_shift_right)
                nc.vector.tensor_scalar(yu, yu, -1, None, ALU.bitwise_xor)
                nc.vector.tensor_scalar(yu, yu, 0x5F3759E0, None, ALU.add)
                rstd = yv
                for _it in range(1):
                    aa = stat_p.tile([P, 1], F32, tag="nsa", name="nsa")
                    nc.vector.tensor_mul(aa, rstd, rstd)
                    nc.vector.tensor_mul(aa, aa, srt)
                    nc.vector.tensor_scalar_add(aa, aa, -3.0)
                    nc.vector.scalar_tensor_tensor(
                        out=rstd, in0=rstd, scalar=-0.5, in1=aa,
                        op0=ALU.mult, op1=ALU.mult,
                    )
                if trivial:
                    if on_act:
                        # (x - mu) * rstd on ACT: x*rstd + (-mu*rstd)
                        nm = stat_p.tile([P, 1], F32, tag="nsa", name="nm")
                        nc.vector.scalar_tensor_tensor(
                            out=nm, in0=mv[:, 0:1], scalar=-1.0, in1=rstd,
                            op0=ALU.mult, op1=ALU.mult,
                        )
                        nc.scalar.activation(dst, src, AF.Identity,
                                             bias=nm, scale=rstd)
                    else:
                        nc.vector.tensor_scalar(
                            dst, src, mv[:, 0:1], rstd,
                            ALU.subtract, ALU.mult,
                        )
                    return
                xc = tail_p.tile([P, D], F32, tag="xc", name="xc")
                nc.vector.tensor_scalar_sub(xc, src, mv[:, 0:1])
                nc.vector.scalar_tensor_tensor(
                    out=dst, in0=xc, scalar=rstd, in1=gb,
                    op0=ALU.mult, op1=ALU.mult,
                )
                nc.vector.tensor_add(dst, dst, bb)

            def tail_res(rb, rc, opre, tag="ln0", on_act=False):
                """residual add + first layernorm; returns the ln0 tile"""
                gr = rb * (RB // P) + rc
                x = opre[rc].rearrange("p h d -> p (h d)")
                for kc in range(NKC):
                    pt = ps_sc.tile([P, P], F32, tag="psc", name="ptr")
                    nc.tensor.transpose(
                        pt, qT.bitcast(F32)[:, kc, gr * P:(gr + 1) * P], ident
                    )
                    nc.vector.tensor_add(
                        x[:, kc * P:(kc + 1) * P],
                        x[:, kc * P:(kc + 1) * P], pt
                    )
                ln0 = tail_p.tile([P, D], F32, tag=tag, name="ln0")
                layernorm(ln0, x, g0b, b0b, on_act=on_act)
                return ln0

            def tail_mlp(rb, rc, ln0, on_act=False):
                gr = rb * (RB // P) + rc
                lnT = tail_p.tile([P, NKC, P], F32R, tag="lnT", name="lnT")
                for kc in range(NKC):
                    pt = ps_sc.tile([P, P], F32, tag="psc", name="ptl")
                    nc.tensor.transpose(pt, ln0[:, kc * P:(kc + 1) * P], ident)
                    if on_act:
                        nc.scalar.activation(lnT[:, kc], pt, AF.Copy)
                    else:
                        nc.vector.tensor_copy(lnT[:, kc], pt)
                pm = ps_sc.tile([P, D], F32, tag="psc", name="pmo")
                for kc in range(NKC):
                    nc.tensor.matmul(
                        pm, lhsT=lnT[:, kc], rhs=wo_t[:, kc],
                        start=(kc == 0), stop=(kc == NKC - 1),
                    )
                mlp = tail_p.tile([P, D], F32, tag="xc", name="mlp_t")
                if trivial:
                    if on_act:
                        nc.scalar.activation(mlp, pm, AF.Relu)
                    else:
                        nc.vector.tensor_scalar_max(mlp, pm, 0.0)
                else:
                    nc.vector.scalar_tensor_tensor(
                        out=mlp, in0=pm, scalar=1.0, in1=bob,
                        op0=ALU.mult, op1=ALU.add,
                    )
                    nc.vector.tensor_scalar_max(mlp, mlp, 0.0)
                nc.vector.tensor_add(mlp, mlp, ln0)
                out_t = tail_p.tile([P, D], F32, tag="xc", name="out_t")
                layernorm(out_t, mlp, g1b, b1b, on_act=on_act)
                nc.sync.dma_start(Oo[gr * P:(gr + 1) * P, :], out_t)

            # ---- phase A interleaved with C(rb=0, pair=0) ----
            wv_t = w_p.tile([P, NKC, D], F32R, tag="w", name="wv_t")
            nc.gpsimd.dma_start(wv_t, Wv.rearrange("(c p) n -> p c n", p=P))

            opre0 = [opre_p.tile([P, H, DH], F32, tag="opre", name=f"opre0_{i}")
                     for i in range(RB // P)]
            opre1 = [opre_p.tile([P, H, DH], F32, tag="opre", name=f"opre1_{i}")
                     for i in range(RB // P)]
            pend = []  # rolling software-pipeline queue across pairs

            def c_emit(rb, pair, i, sp, obig, opre):
                post = None
                if sp == NSP - 1 and i == 1:
                    post = (lambda rb=rb, pair=pair, ob=obig, op=opre:
                            c_post(rb, pair, ob, op))
                pend.append(c_scores(rb, pair, i, sp) + (obig, post))
                if len(pend) > PV_LAG:
                    c_pv(pend.pop(0))

            LAG = 1
            SPP = SC2 // P // 2  # key scpairs produced per a_step (2)
            obig = ps_o.tile([P, 2, RB], F32, tag="ob", name="oacc0")
            for sc2 in range(S // SC2):
                a_step(sc2, pre=KTc0 if sc2 == 0 else None)
                if sc2 >= LAG:
                    for sp in range(SPP * (sc2 - LAG), SPP * (sc2 - LAG) + SPP):
                        for i in (0, 1):
                            c_emit(0, 0, i, sp, obig, opre0)
            for sp in range(SPP * (S // SC2 - LAG), NSP):
                for i in (0, 1):
                    c_emit(0, 0, i, sp, obig, opre0)

            # ---- remaining rb0 pairs (qT rb1-half emitted inside pair1) ----
            for pair in range(1, H // 2):
                obig = ps_o.tile([P, 2, RB], F32, tag="ob", name=f"oacc_r0_p{pair}")
                for sp in range(NSP):
                    for i in (0, 1):
                        c_emit(0, pair, i, sp, obig, opre0)
                    if sp == 2:
                        c_post_finish()
                    if pair == 1 and sp == 6:
                        b_step(1)

            wo_t = w_p.tile([P, NKC, D], F32R, tag="w", name="wo_t")
            nc.gpsimd.dma_start(wo_t, Wo.rearrange("(c p) n -> p c n", p=P))

            # ---- rb1 pairs with rb0 tails spread into their PE slack ----
            for pair in range(H // 2):
                obig = ps_o.tile([P, 2, RB], F32, tag="ob", name=f"oacc_r1_p{pair}")
                ln0_s = None
                for sp in range(NSP):
                    for i in (0, 1):
                        c_emit(1, pair, i, sp, obig, opre1)
                    if sp == 2:
                        c_post_finish()
                    elif sp == 5:
                        ln0_s = tail_res(0, pair, opre0)
                    elif sp == 10:
                        tail_mlp(0, pair, ln0_s)
            while pend:
                c_pv(pend.pop(0))
            c_post_finish()
            # final 4 row-chunks: interleave stages across chunks so the
            # layernorm latency chains overlap with PE work of other chunks;
            # ACT (done with exp by now) takes the normalize/relu/copies
            ln0_f = [tail_res(1, rc, opre1, tag=f"lnf{rc}", on_act=True)
                     for rc in range(RB // P)]
            for rc in range(RB // P):
                tail_mlp(1, rc, ln0_f[rc], on_act=True)

    nsplit, nnops = _split_multi_waits(nc)
    print(f"wait-split: {nsplit} instructions, {nnops} nops inserted")
    return nc


_cached = {}


def _get_nc(trivial):
    if trivial not in _cached:
        _cached[trivial] = build_nc(trivial=trivial)
    return _cached[trivial]


def kernel(Q, K, Wq, bq, Wk, bk, Wv, bv, Wo, bo, g0, b0, g1, b1):
    from concourse.bass_utils import run_bass_kernel_spmd

    trivial = all(
        np.all(np.asarray(v) == c)
        for v, c in ((bq, 0), (bk, 0), (bv, 0), (bo, 0),
                     (g0, 1), (b0, 0), (g1, 1), (b1, 0))
    )
    nc = _get_nc(trivial)
    Q = np.ascontiguousarray(Q, dtype=np.float32)
    K = np.ascontiguousarray(K, dtype=np.float32)
    shared = {
        "Wq": np.ascontiguousarray(Wq, np.float32),
        "Wk": np.ascontiguousarray(Wk, np.float32),
        "Wv": np.ascontiguousarray(Wv, np.float32),
        "Wo": np.ascontiguousarray(Wo, np.float32),
        "bq": np.ascontiguousarray(bq, np.float32),
        "bk": np.ascontiguousarray(bk, np.float32),
        "bv": np.ascontiguousarray(bv, np.float32),
        "bo": np.ascontiguousarray(bo, np.float32),
        "g0": np.ascontiguousarray(g0, np.float32),
        "b0": np.ascontiguousarray(b0, np.float32),
        "g1": np.ascontiguousarray(g1, np.float32),
        "b1": np.ascontiguousarray(b1, np.float32),
    }
    in_maps = []
    KT_by_b = [np.ascontiguousarray(K[b].T) for b in range(B)]
    for c in range(NC):
        b, roff = c // 4, (c % 4) * RPC
        in_maps.append(
            dict(shared,
                 QT=np.ascontiguousarray(Q[b, roff:roff + RPC].T),
                 KTb=KT_by_b[b])
        )
    res = run_bass_kernel_spmd(nc, in_maps, core_ids=list(range(NC)))
    out = np.empty((B, S, D), np.float32)
    for c in range(NC):
        b, roff = c // 4, (c % 4) * RPC
        out[b, roff:roff + RPC] = res.results[c]["O"]
    return out



# revision 36
# speedup vs baseline: 1.2539x; 1.2539x over previous
"""Trainium2 Bass kernel: transformer block (QKV proj + MHA + residual + LN +
MLP(relu) residual + LN) for B=2, S=4096, D=512, H=8.

Sharding: data-parallel over (batch, query-row-block) — 8 cores x 1024 query
rows. Each core recomputes K/V projections for its batch (4 cores share a
batch), attends over all 4096 keys, and runs the per-row tail. No cross-core
communication.

Layouts: feature-major ("T" = [d, rows]) so projection/attention matmuls chain
without re-transposing. Softmax sums come free from a ones-column appended to
each V tile. fp32r matmuls (full PE rate at moving-dim >= 256).
"""

import math

import numpy as np

import concourse.bass as bass
import concourse.mybir as mybir
import concourse.tile as tile
from concourse.masks import make_identity

# ---------------------------------------------------------------------------
# Workaround: this walrus build rejects >1 sync-wait on the TileContext exit
# drain (CoreV3 setupSyncWait "Too many sync wait commands"). Split the waits
# across single-wait NOPs.
_orig_drain_and_barrier = tile.TileContext._drain_and_barrier


def _split_drain_and_barrier(self, tick_clock, wait_clock):
    from concourse.tile import ScopedClock

    nc = self.nc
    drain_inst = nc.sync.drain()
    wait_clock.add_sem_waits(
        drain_inst.ins, ScopedClock({None: tick_clock.global_clock})
    )
    si = drain_inst.ins.sync_info
    waits = list(si.on_wait) if si and si.on_wait else []
    if len(waits) > 1:
        si.on_wait = waits[:1]
        for w in waits[1:]:
            nop = nc.sync.nop(nofuse=True, hint="drain_wait_split")
            nop.ins.sync_info = mybir.SyncInfo(on_wait=[w], on_update=[])
    nc.all_engine_barrier()
    assert self.sems is not None
    popped = nc._tile_sem_poison_stack.pop()
    assert popped is self._sem_poison
    nc.clear_and_free_semaphores(list(self.sems.allocated().values()))
    nc.all_engine_barrier()


tile.TileContext._drain_and_barrier = _split_drain_and_barrier


def _split_multi_waits(nc, limit=1):
    """walrus CoreV3 codegen caps sync-waits per instruction descriptor; hoist
    excess waits onto fresh NOPs inserted just before the instruction on the
    same engine queue. Matmul (S3_LW) and Drain (CTRL_NO) descriptors only
    fit 1 wait; other engine descriptors fit 2."""
    ctr = [0]

    def mknop(engine, wait):
        ctr[0] += 1
        nop = mybir.InstNoOp(name=f"WSPLIT-{ctr[0]}", ins=[], outs=[])
        nop.engine = engine
        nop.sync_info = mybir.SyncInfo(on_wait=[wait], on_update=[])
        return nop

    nsplit = 0
    for f in nc.m.functions:
        for bb in f.blocks:
            insts = bb.instructions
            i = 0
            while i < len(insts):
                ins = insts[i]
                si = getattr(ins, "sync_info", None)
                if si is not None and si.on_wait and len(si.on_wait) > limit:
                    waits = list(si.on_wait)
                    si.on_wait = waits[-limit:]
                    pre = [mknop(ins.engine, w) for w in waits[:-limit]]
                    for j, p in enumerate(pre):
                        insts.insert(i + j, p)
                        nc.register_instruction(p, overwrite=True)
                    i += len(pre)
                    nsplit += 1
                i += 1
    return nsplit, ctr[0]
# ---------------------------------------------------------------------------

B, S, D, H, DH = 2, 4096, 512, 8, 64
P = 128
NC = 8          # cores
RPC = 1024      # query rows per core
NKC = D // P    # 4 contraction chunks of 128
EPS = 1e-5
SCALE = 1.0 / math.sqrt(D)

F32 = mybir.dt.float32
F32R = mybir.dt.float32r
BF16 = mybir.dt.bfloat16
FP8 = mybir.dt.float8e4
I32 = mybir.dt.int32
U8 = mybir.dt.uint8
ALU = mybir.AluOpType
AF = mybir.ActivationFunctionType

# Schraudolph exp for fp8e4 bit patterns: bits = round(8*log2(exp(s*SCALE)))+56
# = s * (8*log2(e)*SCALE) + 56 (+ rounding-bias correction). Max rel err ~7%
# in the normal range; only the (negligible-weight) denormal tail is worse.
A_EXP = 8.0 * math.log2(math.e) * SCALE
B_EXP = 55.55
# every DVE_MOD-th c-block's exp runs on DVE instead of ACT
DVE_MOD = 1000000


def r(ap):
    """view an fp32 AP as float32r for full-rate PE matmuls"""
    return ap.bitcast(F32R)


def build_nc(reps=1, trivial=True):
    """trivial=True assumes zero q/k/v/o biases and identity layernorm
    params (g=1, b=0) — checked at runtime in kernel(), which falls back
    to the general variant otherwise."""
    nc = bass.Bass()

    # host feeds feature-major (pre-transposed) Q and K slices
    QT = nc.dram_tensor("QT", [D, RPC], F32R, kind="ExternalInput")
    KTb = nc.dram_tensor("KTb", [D, S], F32R, kind="ExternalInput")
    Wq = nc.dram_tensor("Wq", [D, D], F32R, kind="ExternalInput")
    Wk = nc.dram_tensor("Wk", [D, D], F32R, kind="ExternalInput")
    Wv = nc.dram_tensor("Wv", [D, D], F32R, kind="ExternalInput")
    Wo = nc.dram_tensor("Wo", [D, D], F32R, kind="ExternalInput")
    bq = nc.dram_tensor("bq", [D], F32, kind="ExternalInput")
    bk = nc.dram_tensor("bk", [D], F32, kind="ExternalInput")
    bv = nc.dram_tensor("bv", [D], F32, kind="ExternalInput")
    bo = nc.dram_tensor("bo", [D], F32, kind="ExternalInput")
    g0 = nc.dram_tensor("g0", [D], F32, kind="ExternalInput")
    b0 = nc.dram_tensor("b0", [D], F32, kind="ExternalInput")
    g1 = nc.dram_tensor("g1", [D], F32, kind="ExternalInput")
    b1 = nc.dram_tensor("b1", [D], F32, kind="ExternalInput")
    Oo = nc.dram_tensor("O", [RPC, D], F32, kind="ExternalOutput")

    def bcast_ap(dram_vec):
        # [cols] dram vector -> [P, cols] partition-broadcast AP
        a = dram_vec[:]
        return bass.AP(
            tensor=a.tensor,
            offset=a.offset,
            ap=[[0, P]] + list(a.ap),
        )

    def chunked_ap(dram_vec):
        # [D] dram vector -> [P, NKC]: partition = idx within 128-chunk
        return dram_vec.rearrange("(c p) -> p c", p=P)

    with tile.TileContext(nc) as tc:
        with (
            tc.tile_pool(name="consts", bufs=1) as consts,
            tc.tile_pool(name="kT_p", bufs=1) as kT_p,
            tc.tile_pool(name="vx_p", bufs=1) as vx_p,
            tc.tile_pool(name="qT_p", bufs=1) as qT_p,
            tc.tile_pool(name="w_p", bufs=3) as w_p,
            tc.tile_pool(name="ktc_p", bufs=2) as ktc_p,
            tc.tile_pool(name="kload_p", bufs=2) as kload_p,
            tc.tile_pool(name="at_p", bufs=7) as at_p,
            tc.tile_pool(name="ot_p", bufs=2) as ot_p,
            tc.tile_pool(name="opre_p", bufs=7) as opre_p,
            tc.tile_pool(name="tail_p", bufs=2) as tail_p,
            tc.tile_pool(name="stat_p", bufs=4) as stat_p,
            tc.tile_pool(name="ps_sc", bufs=3, space="PSUM") as ps_sc,
            tc.tile_pool(name="ps_o", bufs=1, space="PSUM") as ps_o,
        ):
          for _rep in range(reps):
            # ---- constants ----
            ident = consts.tile([P, P], F32)
            make_identity(nc, ident)

            SC2 = 512
            RB = 512

            # weights + first QTc/KTc loaded in interleaved 256KB chunks so
            # the first projection matmuls start as early as possible
            wq_t = w_p.tile([P, NKC, D], F32R, tag="w", name="wq_t")
            wk_t = w_p.tile([P, NKC, D], F32R, tag="w", name="wk_t")
            QTc0 = ktc_p.tile([P, NKC, SC2], F32R, tag="ktc", name="QTc0")
            KTc0 = ktc_p.tile([P, NKC, SC2], F32R, tag="ktc", name="KTc0")
            wq_d = Wq.rearrange("(c p) n -> p c n", p=P)
            wk_d = Wk.rearrange("(c p) n -> p c n", p=P)
            qt_d = QT.rearrange("(c p) r -> p c r", p=P)
            kt_d = KTb.rearrange("(c p) r -> p c r", p=P)
            for kc in range(NKC):
                nc.sync.dma_start(wq_t[:, kc], wq_d[:, kc])
                nc.sync.dma_start(QTc0[:, kc], qt_d[:, kc, 0:SC2])
            for kc in range(NKC):
                nc.gpsimd.dma_start(wk_t[:, kc], wk_d[:, kc])
                nc.gpsimd.dma_start(KTc0[:, kc], kt_d[:, kc, 0:SC2])
            bias_sb = consts.tile([P, 3, NKC], F32)
            if not trivial:
                for wi, bvec in enumerate((bq, bk, bv)):
                    nc.gpsimd.dma_start(bias_sb[:, wi], chunked_ap(bvec))
            bvb = consts.tile([P, H, DH], F32)
            bob = consts.tile([P, D], F32)
            g0b = consts.tile([P, D], F32)
            b0b = consts.tile([P, D], F32)
            g1b = consts.tile([P, D], F32)
            b1b = consts.tile([P, D], F32)
            if not trivial:
                nc.gpsimd.dma_start(
                    bvb, bcast_ap(bv).rearrange("p (h d) -> p h d", h=H))
                for t, v in ((bob, bo), (g0b, g0), (b0b, b0),
                             (g1b, g1), (b1b, b1)):
                    nc.gpsimd.dma_start(t, bcast_ap(v))

            # ---- persistent activations ----
            kT = kT_p.tile([P, NKC, S], F32R)          # (K Wk + bk)^T
            # V rows (fp8) + ones col at DH + zero pad at DH+1 so the
            # DoubleRow Ko stride (H*(DH+2) = 528 B) is 16-byte aligned
            v_ext = vx_p.tile([P, S // P, H, DH + 2], FP8)
            qT = qT_p.tile([P, NKC, RPC], F32R)        # (Q Wq + bq)^T
            nc.vector.memset(v_ext[:, :, :, DH:DH + 1], 1.0)
            nc.vector.memset(v_ext[:, :, :, DH + 1:DH + 2], 0.0)

            def psum_drain(dst, pp, bias_ap):
                """psum -> SBUF with optional per-partition bias add"""
                if trivial:
                    nc.vector.tensor_copy(dst, pp)
                else:
                    nc.vector.tensor_scalar_add(dst, pp, bias_ap)

            # ---- phase B: qT = Wq^T Q^T + bq (Q^T DMA'd directly) ----
            def b_step(rc2, pre=None):
                if pre is None:
                    QTc = ktc_p.tile([P, NKC, SC2], F32R, tag="ktc", name="QTc")
                    nc.sync.dma_start(QTc, qt_d[:, :, rc2 * SC2:(rc2 + 1) * SC2])
                else:
                    QTc = pre
                for ci in range(NKC):
                    pp = ps_sc.tile([P, SC2], F32, tag="psc", name="ppq")
                    for kc in range(NKC):
                        nc.tensor.matmul(
                            pp,
                            lhsT=wq_t[:, kc, ci * P:(ci + 1) * P],
                            rhs=QTc[:, kc],
                            start=(kc == 0), stop=(kc == NKC - 1),
                        )
                    psum_drain(qT[:, ci, rc2 * SC2:(rc2 + 1) * SC2], pp,
                               bias_sb[:, 0, ci:ci + 1])

            b_step(0, pre=QTc0)

            # ---- helpers ----
            def a_step(sc2, pre=None):
                """project K cols [sc2*512, (sc2+1)*512) into kT and v_ext"""
                if pre is None:
                    KTc = ktc_p.tile([P, NKC, SC2], F32R, tag="ktc", name="KTc")
                    nc.sync.dma_start(KTc, kt_d[:, :, sc2 * SC2:(sc2 + 1) * SC2])
                else:
                    KTc = pre
                for ci in range(NKC):
                    pp = ps_sc.tile([P, SC2], F32, tag="psc", name="ppk")
                    for kc in range(NKC):
                        nc.tensor.matmul(
                            pp,
                            lhsT=wk_t[:, kc, ci * P:(ci + 1) * P],
                            rhs=KTc[:, kc],
                            start=(kc == 0), stop=(kc == NKC - 1),
                        )
                    psum_drain(kT[:, ci, sc2 * SC2:(sc2 + 1) * SC2], pp,
                               bias_sb[:, 1, ci:ci + 1])
                for half in range(SC2 // P):
                    pv = ps_sc.tile([P, D], F32, tag="psc", name="ppv")
                    for kc in range(NKC):
                        nc.tensor.matmul(
                            pv,
                            lhsT=KTc[:, kc, half * P:(half + 1) * P],
                            rhs=wv_t[:, kc],
                            start=(kc == 0), stop=(kc == NKC - 1),
                        )
                    sidx = sc2 * (SC2 // P) + half
                    pvh = pv.rearrange("p (h d) -> p h d", h=H)
                    if trivial:
                        nc.vector.tensor_copy(v_ext[:, sidx, :, 0:DH], pvh)
                    else:
                        nc.vector.scalar_tensor_tensor(
                            out=v_ext[:, sidx, :, 0:DH],
                            in0=pvh, scalar=1.0, in1=bvb,
                            op0=ALU.mult, op1=ALU.add,
                        )

            NSP = S // P // 2  # 16 key scpairs (256 keys each)
            PV_LAG = 5         # PV matmuls trail scores/exp by this many blocks

            def c_scores(rb, pair, i, scpair):
                """scores+exp for head 2*pair+i over keys
                [scpair*256, scpair*256+256) x queries [rb*RB, rb*RB+RB).
                Returns the fp8 exp tile for the trailing PV matmul."""
                h = 2 * pair + i
                ci, po = h // 2, (h % 2) * DH
                ps = ps_sc.tile([P, 2, RB], F32, tag="psc", name="psc")
                for j in (0, 1):
                    sc = 2 * scpair + j
                    nc.tensor.matmul(
                        ps[:, j],
                        lhsT=kT[po:po + DH, ci, sc * P:(sc + 1) * P],
                        rhs=qT[po:po + DH, ci, rb * RB:(rb + 1) * RB],
                        start=True, stop=True,
                    )
                at = at_p.tile([P, 2, RB], FP8, tag="at", name="at")
                if (scpair * 2 + i) % DVE_MOD == DVE_MOD - 1:
                    nc.vector.tensor_scalar(
                        at.bitcast(U8), ps, A_EXP, B_EXP, ALU.mult, ALU.add
                    )
                else:
                    nc.scalar.activation(at, ps, AF.Exp, scale=SCALE)
                return (at, i, scpair, h)

            def c_pv(blk):
                at, i, scpair, h, obig, post = blk
                nc.tensor.matmul(
                    obig[0:DH + 1, i],
                    lhsT=v_ext[:, 2 * scpair:2 * scpair + 2, h, 0:DH + 1],
                    rhs=at,
                    start=(scpair == 0), stop=(scpair == NSP - 1),
                    perf_mode=mybir.MatmulPerfMode.DoubleRow,
                )
                if post is not None:
                    post()

            deferred_post = []

            def c_post(rb, pair, obig, opre):
                # copy obig out of PSUM now (frees the accumulator slot);
                # defer the transpose+normalize (which needs psum slots) so
                # it doesn't contend with the next pair's score tiles
                for i, h in enumerate((2 * pair, 2 * pair + 1)):
                    ot = ot_p.tile([DH + 1, RB], F32, tag="ot", name="ot")
                    nc.vector.tensor_copy(ot, obig[0:DH + 1, i])
                    deferred_post.append((ot, h, opre))

            def c_post_finish():
                while deferred_post:
                    ot, h, opre = deferred_post.pop(0)
                    for rc in range(RB // P):
                        pt = ps_sc.tile([P, P], F32, tag="psc", name="pto")
                        nc.tensor.transpose(
                            pt[:, 0:DH + 1], ot[:, rc * P:(rc + 1) * P],
                            ident[0:DH + 1, 0:DH + 1]
                        )
                        rec = stat_p.tile([P, 1], F32, tag="rec", name="rec")
                        nc.vector.reciprocal(rec, pt[:, DH:DH + 1])
                        nc.vector.tensor_scalar_mul(
                            opre[rc][:, h], pt[:, 0:DH], rec
                        )

            def layernorm(dst, src, gb, bb, on_act=False):
                st6 = stat_p.tile([P, 6], F32, tag="st6", name="st6")
                nc.vector.bn_stats(st6, src)
                mv = stat_p.tile([P, 2], F32, tag="mv", name="mv")
                nc.vector.bn_aggr(mv, st6)
                # rstd = rsqrt(var+eps) on DVE: quake seed + 1 Newton step
                # (keeps ACT's table on Exp during the attention stream)
                srt = stat_p.tile([P, 1], F32, tag="srt", name="srt")
                nc.vector.tensor_scalar_add(srt, mv[:, 1:2], EPS)
                yv = stat_p.tile([P, 1], F32, tag="rstd", name="yv")
                yu = yv.bitcast(I32)
                nc.vector.tensor_scalar(yu, srt.bitcast(I32), 1, None,
                                        ALU.logical_shift_right)
                nc.vector.tensor_scalar(yu, yu, -1, None, ALU.bitwise_xor)
                nc.vector.tensor_scalar(yu, yu, 0x5F3759E0, None, ALU.add)
                rstd = yv
                for _it in range(1):
                    aa = stat_p.tile([P, 1], F32, tag="nsa", name="nsa")
                    nc.vector.tensor_mul(aa, rstd, rstd)
                    nc.vector.tensor_mul(aa, aa, srt)
                    nc.vector.tensor_scalar_add(aa, aa, -3.0)
                    nc.vector.scalar_tensor_tensor(
                        out=rstd, in0=rstd, scalar=-0.5, in1=aa,
                        op0=ALU.mult, op1=ALU.mult,
                    )
                if trivial:
                    if on_act:
                        # (x - mu) * rstd on ACT: x*rstd + (-mu*rstd)
                        nm = stat_p.tile([P, 1], F32, tag="nsa", name="nm")
                        nc.vector.scalar_tensor_tensor(
                            out=nm, in0=mv[:, 0:1], scalar=-1.0, in1=rstd,
                            op0=ALU.mult, op1=ALU.mult,
                        )
                        nc.scalar.activation(dst, src, AF.Identity,
                                             bias=nm, scale=rstd)
                    else:
                        nc.vector.tensor_scalar(
                            dst, src, mv[:, 0:1], rstd,
                            ALU.subtract, ALU.mult,
                        )
                    return
                xc = tail_p.tile([P, D], F32, tag="xc", name="xc")
                nc.vector.tensor_scalar_sub(xc, src, mv[:, 0:1])
                nc.vector.scalar_tensor_tensor(
                    out=dst, in0=xc, scalar=rstd, in1=gb,
                    op0=ALU.mult, op1=ALU.mult,
                )
                nc.vector.tensor_add(dst, dst, bb)

            def tail_res(rb, rc, opre, tag="ln0", on_act=False):
                """residual add + first layernorm; returns the ln0 tile"""
                gr = rb * (RB // P) + rc
                x = opre[rc].rearrange("p h d -> p (h d)")
                for kc in range(NKC):
                    pt = ps_sc.tile([P, P], F32, tag="psc", name="ptr")
                    nc.tensor.transpose(
                        pt, qT.bitcast(F32)[:, kc, gr * P:(gr + 1) * P], ident
                    )
                    nc.vector.tensor_add(
                        x[:, kc * P:(kc + 1) * P],
                        x[:, kc * P:(kc + 1) * P], pt
                    )
                ln0 = tail_p.tile([P, D], F32, tag=tag, name="ln0")
                layernorm(ln0, x, g0b, b0b, on_act=on_act)
                return ln0

            def tail_mlp(rb, rc, ln0, on_act=False):
                gr = rb * (RB // P) + rc
                lnT = tail_p.tile([P, NKC, P], F32R, tag="lnT", name="lnT")
                for kc in range(NKC):
                    pt = ps_sc.tile([P, P], F32, tag="psc", name="ptl")
                    nc.tensor.transpose(pt, ln0[:, kc * P:(kc + 1) * P], ident)
                    if on_act:
                        nc.scalar.activation(lnT[:, kc], pt, AF.Copy)
                    else:
                        nc.vector.tensor_copy(lnT[:, kc], pt)
                pm = ps_sc.tile([P, D], F32, tag="psc", name="pmo")
                for kc in range(NKC):
                    nc.tensor.matmul(
                        pm, lhsT=lnT[:, kc], rhs=wo_t[:, kc],
                        start=(kc == 0), stop=(kc == NKC - 1),
                    )
                mlp = tail_p.tile([P, D], F32, tag="xc", name="mlp_t")
                if trivial:
                    if on_act:
                        nc.scalar.activation(mlp, pm, AF.Relu)
                    else:
                        nc.vector.tensor_scalar_max(mlp, pm, 0.0)
                else:
                    nc.vector.scalar_tensor_tensor(
                        out=mlp, in0=pm, scalar=1.0, in1=bob,
                        op0=ALU.mult, op1=ALU.add,
                    )
                    nc.vector.tensor_scalar_max(mlp, mlp, 0.0)
                nc.vector.tensor_add(mlp, mlp, ln0)
                out_t = tail_p.tile([P, D], F32, tag="xc", name="out_t")
                layernorm(out_t, mlp, g1b, b1b, on_act=on_act)
                nc.sync.dma_start(Oo[gr * P:(gr + 1) * P, :], out_t)

            # ---- phase A interleaved with C(rb=0, pair=0) ----
            wv_t = w_p.tile([P, NKC, D], F32R, tag="w", name="wv_t")
            nc.gpsimd.dma_start(wv_t, Wv.rearrange("(c p) n -> p c n", p=P))

            opre0 = [opre_p.tile([P, H, DH], F32, tag="opre", name=f"opre0_{i}")
                     for i in range(RB // P)]
            opre1 = [opre_p.tile([P, H, DH], F32, tag="opre", name=f"opre1_{i}")
                     for i in range(RB // P)]
            pend = []  # rolling software-pipeline queue across pairs

            def c_emit(rb, pair, i, sp, obig, opre):
                post = None
                if sp == NSP - 1 and i == 1:
                    post = (lambda rb=rb, pair=pair, ob=obig, op=opre:
                            c_post(rb, pair, ob, op))
                pend.append(c_scores(rb, pair, i, sp) + (obig, post))
                if len(pend) > PV_LAG:
                    c_pv(pend.pop(0))

            LAG = 1
            SPP = SC2 // P // 2  # key scpairs produced per a_step (2)
            obig = ps_o.tile([P, 2, RB], F32, tag="ob", name="oacc0")
            for sc2 in range(S // SC2):
                a_step(sc2, pre=KTc0 if sc2 == 0 else None)
                if sc2 >= LAG:
                    for sp in range(SPP * (sc2 - LAG), SPP * (sc2 - LAG) + SPP):
                        for i in (0, 1):
                            c_emit(0, 0, i, sp, obig, opre0)
            for sp in range(SPP * (S // SC2 - LAG), NSP):
                for i in (0, 1):
                    c_emit(0, 0, i, sp, obig, opre0)

            # ---- remaining rb0 pairs (qT rb1-half emitted inside pair1) ----
            for pair in range(1, H // 2):
                obig = ps_o.tile([P, 2, RB], F32, tag="ob", name=f"oacc_r0_p{pair}")
                for sp in range(NSP):
                    for i in (0, 1):
                        c_emit(0, pair, i, sp, obig, opre0)
                    if sp == 2:
                        c_post_finish()
                    if pair == 1 and sp == 6:
                        b_step(1)

            wo_t = w_p.tile([P, NKC, D], F32R, tag="w", name="wo_t")
            nc.gpsimd.dma_start(wo_t, Wo.rearrange("(c p) n -> p c n", p=P))

            # ---- rb1 pairs with rb0 tails spread into their PE slack ----
            for pair in range(H // 2):
                obig = ps_o.tile([P, 2, RB], F32, tag="ob", name=f"oacc_r1_p{pair}")
                ln0_s = None
                for sp in range(NSP):
                    for i in (0, 1):
                        c_emit(1, pair, i, sp, obig, opre1)
                    if sp == 2:
                        c_post_finish()
                    elif sp == 5:
                        ln0_s = tail_res(0, pair, opre0)
                    elif sp == 10:
                        tail_mlp(0, pair, ln0_s)
            while pend:
                c_pv(pend.pop(0))
            c_post_finish()
            # final 4 row-chunks: interleave stages across chunks so the
            # layernorm latency chains overlap with PE work of other chunks;
            # ACT (done with exp by now) takes the normalize/relu/copies
            ln0_f = [tail_res(1, rc, opre1, tag=f"lnf{rc}", on_act=True)
                     for rc in range(RB // P)]
            for rc in range(RB // P):
                tail_mlp(1, rc, ln0_f[rc], on_act=True)

    nsplit, nnops = _split_multi_waits(nc)
    print(f"wait-split: {nsplit} instructions, {nnops} nops inserted")
    return nc


_cached = {}


def _get_nc(trivial):
    if trivial not in _cached:
        _cached[trivial] = build_nc(trivial=trivial)
    return _cached[trivial]


def kernel(Q, K, Wq, bq, Wk, bk, Wv, bv, Wo, bo, g0, b0, g1, b1):
    from concourse.bass_utils import run_bass_kernel_spmd

    trivial = all(
        np.all(np.asarray(v) == c)
        for v, c in ((bq, 0), (bk, 0), (bv, 0), (bo, 0),
                     (g0, 1), (b0, 0), (g1, 1), (b1, 0))
    )
    nc = _get_nc(trivial)
    Q = np.ascontiguousarray(Q, dtype=np.float32)
    K = np.ascontiguousarray(K, dtype=np.float32)
    shared = {
        "Wq": np.ascontiguousarray(Wq, np.float32),
        "Wk": np.ascontiguousarray(Wk, np.float32),
        "Wv": np.ascontiguousarray(Wv, np.float32),
        "Wo": np.ascontiguousarray(Wo, np.float32),
        "bq": np.ascontiguousarray(bq, np.float32),
        "bk": np.ascontiguousarray(bk, np.float32),
        "bv": np.ascontiguousarray(bv, np.float32),
        "bo": np.ascontiguousarray(bo, np.float32),
        "g0": np.ascontiguousarray(g0, np.float32),
        "b0": np.ascontiguousarray(b0, np.float32),
        "g1": np.ascontiguousarray(g1, np.float32),
        "b1": np.ascontiguousarray(b1, np.float32),
    }
    in_maps = []
    KT_by_b = [np.ascontiguousarray(K[b].T) for b in range(B)]
    for c in range(NC):
        b, roff = c // 4, (c % 4) * RPC
        in_maps.append(
            dict(shared,
                 QT=np.ascontiguousarray(Q[b, roff:roff + RPC].T),
                 KTb=KT_by_b[b])
        )
    res = run_bass_kernel_spmd(nc, in_maps, core_ids=list(range(NC)))
    out = np.empty((B, S, D), np.float32)
    for c in range(NC):
        b, roff = c // 4, (c % 4) * RPC
        out[b, roff:roff + RPC] = res.results[c]["O"]
    return out



# revision 41
# speedup vs baseline: 1.3877x; 1.1067x over previous
"""Trainium2 Bass kernel: transformer block (QKV proj + MHA + residual + LN +
MLP(relu) residual + LN) for B=2, S=4096, D=512, H=8.

Sharding: data-parallel over (batch, query-row-block) — 8 cores x 1024 query
rows. Each core recomputes K/V projections for its batch (4 cores share a
batch), attends over all 4096 keys, and runs the per-row tail. No cross-core
communication.

Layouts: feature-major ("T" = [d, rows]) so projection/attention matmuls chain
without re-transposing. Softmax sums come free from a ones-column appended to
each V tile. fp32r matmuls (full PE rate at moving-dim >= 256).
"""

import math

import numpy as np

import concourse.bass as bass
import concourse.mybir as mybir
import concourse.tile as tile
from concourse.masks import make_identity

# ---------------------------------------------------------------------------
# Workaround: this walrus build rejects >1 sync-wait on the TileContext exit
# drain (CoreV3 setupSyncWait "Too many sync wait commands"). Split the waits
# across single-wait NOPs.
_orig_drain_and_barrier = tile.TileContext._drain_and_barrier


def _split_drain_and_barrier(self, tick_clock, wait_clock):
    from concourse.tile import ScopedClock

    nc = self.nc
    drain_inst = nc.sync.drain()
    wait_clock.add_sem_waits(
        drain_inst.ins, ScopedClock({None: tick_clock.global_clock})
    )
    si = drain_inst.ins.sync_info
    waits = list(si.on_wait) if si and si.on_wait else []
    if len(waits) > 1:
        si.on_wait = waits[:1]
        for w in waits[1:]:
            nop = nc.sync.nop(nofuse=True, hint="drain_wait_split")
            nop.ins.sync_info = mybir.SyncInfo(on_wait=[w], on_update=[])
    nc.all_engine_barrier()
    assert self.sems is not None
    popped = nc._tile_sem_poison_stack.pop()
    assert popped is self._sem_poison
    nc.clear_and_free_semaphores(list(self.sems.allocated().values()))
    nc.all_engine_barrier()


tile.TileContext._drain_and_barrier = _split_drain_and_barrier


def _split_multi_waits(nc, limit=1):
    """walrus CoreV3 codegen caps sync-waits per instruction descriptor; hoist
    excess waits onto fresh NOPs inserted just before the instruction on the
    same engine queue. Matmul (S3_LW) and Drain (CTRL_NO) descriptors only
    fit 1 wait; other engine descriptors fit 2."""
    ctr = [0]

    def mknop(engine, wait):
        ctr[0] += 1
        nop = mybir.InstNoOp(name=f"WSPLIT-{ctr[0]}", ins=[], outs=[])
        nop.engine = engine
        nop.sync_info = mybir.SyncInfo(on_wait=[wait], on_update=[])
        return nop

    nsplit = 0
    for f in nc.m.functions:
        for bb in f.blocks:
            insts = bb.instructions
            i = 0
            while i < len(insts):
                ins = insts[i]
                si = getattr(ins, "sync_info", None)
                if si is not None and si.on_wait and len(si.on_wait) > limit:
                    waits = list(si.on_wait)
                    si.on_wait = waits[-limit:]
                    pre = [mknop(ins.engine, w) for w in waits[:-limit]]
                    for j, p in enumerate(pre):
                        insts.insert(i + j, p)
                        nc.register_instruction(p, overwrite=True)
                    i += len(pre)
                    nsplit += 1
                i += 1
    return nsplit, ctr[0]
# ---------------------------------------------------------------------------

B, S, D, H, DH = 2, 4096, 512, 8, 64
P = 128
NC = 8          # cores
RPC = 1024      # query rows per core
NKC = D // P    # 4 contraction chunks of 128
EPS = 1e-5
SCALE = 1.0 / math.sqrt(D)

F32 = mybir.dt.float32
F32R = mybir.dt.float32r
BF16 = mybir.dt.bfloat16
FP8 = mybir.dt.float8e4
I32 = mybir.dt.int32
U8 = mybir.dt.uint8
ALU = mybir.AluOpType
AF = mybir.ActivationFunctionType

# Schraudolph exp for fp8e4 bit patterns: bits = round(8*log2(exp(s*SCALE)))+56
# = s * (8*log2(e)*SCALE) + 56 (+ rounding-bias correction). Max rel err ~7%
# in the normal range; only the (negligible-weight) denormal tail is worse.
A_EXP = 8.0 * math.log2(math.e) * SCALE
B_EXP = 55.55
# every DVE_MOD-th c-block's exp runs on DVE instead of ACT
DVE_MOD = 4


def r(ap):
    """view an fp32 AP as float32r for full-rate PE matmuls"""
    return ap.bitcast(F32R)


SQ = S // 4  # keys projected locally per core in cc mode


def build_nc(reps=1, trivial=True, cc=True):
    """trivial=True assumes zero q/k/v/o biases and identity layernorm
    params (g=1, b=0) — checked at runtime in kernel(), which falls back
    to the general variant otherwise.
    cc=True shards the K/V projection across the 4 cores of each batch
    (each projects a quarter of the keys) and AllGathers kT/v_ext."""
    nc = bass.Bass(num_devices=NC)

    # host feeds feature-major (pre-transposed) Q and K slices
    QT = nc.dram_tensor("QT", [D, RPC], F32R, kind="ExternalInput")
    KTb = nc.dram_tensor("KTb", [D, SQ if cc else S], F32R,
                         kind="ExternalInput")
    if cc:
        VROW = H * (DH + 2)
        cck_i = nc.dram_tensor("cck_i", [D, SQ], F32R)
        cck_o = nc.dram_tensor("cck_o", [4, D, SQ], F32R)
        ccv_i = nc.dram_tensor("ccv_i", [P, SQ // P, VROW], U8)
        ccv_o = nc.dram_tensor("ccv_o", [4, P, SQ // P, VROW], U8)
        CC_GROUPS = [[0, 1, 2, 3], [4, 5, 6, 7]]
    Wq = nc.dram_tensor("Wq", [D, D], F32R, kind="ExternalInput")
    Wk = nc.dram_tensor("Wk", [D, D], F32R, kind="ExternalInput")
    Wv = nc.dram_tensor("Wv", [D, D], F32R, kind="ExternalInput")
    Wo = nc.dram_tensor("Wo", [D, D], F32R, kind="ExternalInput")
    bq = nc.dram_tensor("bq", [D], F32, kind="ExternalInput")
    bk = nc.dram_tensor("bk", [D], F32, kind="ExternalInput")
    bv = nc.dram_tensor("bv", [D], F32, kind="ExternalInput")
    bo = nc.dram_tensor("bo", [D], F32, kind="ExternalInput")
    g0 = nc.dram_tensor("g0", [D], F32, kind="ExternalInput")
    b0 = nc.dram_tensor("b0", [D], F32, kind="ExternalInput")
    g1 = nc.dram_tensor("g1", [D], F32, kind="ExternalInput")
    b1 = nc.dram_tensor("b1", [D], F32, kind="ExternalInput")
    Oo = nc.dram_tensor("O", [RPC, D], F32, kind="ExternalOutput")

    def bcast_ap(dram_vec):
        # [cols] dram vector -> [P, cols] partition-broadcast AP
        a = dram_vec[:]
        return bass.AP(
            tensor=a.tensor,
            offset=a.offset,
            ap=[[0, P]] + list(a.ap),
        )

    def chunked_ap(dram_vec):
        # [D] dram vector -> [P, NKC]: partition = idx within 128-chunk
        return dram_vec.rearrange("(c p) -> p c", p=P)

    with tile.TileContext(nc) as tc:
        with (
            tc.tile_pool(name="consts", bufs=1) as consts,
            tc.tile_pool(name="kT_p", bufs=1) as kT_p,
            tc.tile_pool(name="vx_p", bufs=1) as vx_p,
            tc.tile_pool(name="qT_p", bufs=1) as qT_p,
            tc.tile_pool(name="w_p", bufs=3) as w_p,
            tc.tile_pool(name="ktc_p", bufs=2) as ktc_p,
            tc.tile_pool(name="kload_p", bufs=2) as kload_p,
            tc.tile_pool(name="at_p", bufs=7) as at_p,
            tc.tile_pool(name="ot_p", bufs=2) as ot_p,
            tc.tile_pool(name="opre_p", bufs=7) as opre_p,
            tc.tile_pool(name="tail_p", bufs=2) as tail_p,
            tc.tile_pool(name="stat_p", bufs=4) as stat_p,
            tc.tile_pool(name="ps_sc", bufs=3, space="PSUM") as ps_sc,
            tc.tile_pool(name="ps_o", bufs=1, space="PSUM") as ps_o,
        ):
          for _rep in range(reps):
            # ---- constants ----
            ident = consts.tile([P, P], F32)
            make_identity(nc, ident)

            SC2 = 512
            RB = 512

            # weights + first QTc/KTc loaded in interleaved 256KB chunks so
            # the first projection matmuls start as early as possible
            wq_t = w_p.tile([P, NKC, D], F32R, tag="w", name="wq_t")
            wk_t = w_p.tile([P, NKC, D], F32R, tag="w", name="wk_t")
            QTc0 = ktc_p.tile([P, NKC, SC2], F32R, tag="ktc", name="QTc0")
            KTc0 = ktc_p.tile([P, NKC, SC2], F32R, tag="ktc", name="KTc0")
            wq_d = Wq.rearrange("(c p) n -> p c n", p=P)
            wk_d = Wk.rearrange("(c p) n -> p c n", p=P)
            qt_d = QT.rearrange("(c p) r -> p c r", p=P)
            kt_d = KTb.rearrange("(c p) r -> p c r", p=P)
            for kc in range(NKC):
                nc.sync.dma_start(wq_t[:, kc], wq_d[:, kc])
                nc.sync.dma_start(QTc0[:, kc], qt_d[:, kc, 0:SC2])
            for kc in range(NKC):
                nc.gpsimd.dma_start(wk_t[:, kc], wk_d[:, kc])
                nc.gpsimd.dma_start(KTc0[:, kc], kt_d[:, kc, 0:SC2])
            bias_sb = consts.tile([P, 3, NKC], F32)
            if not trivial:
                for wi, bvec in enumerate((bq, bk, bv)):
                    nc.gpsimd.dma_start(bias_sb[:, wi], chunked_ap(bvec))
            bvb = consts.tile([P, H, DH], F32)
            bob = consts.tile([P, D], F32)
            g0b = consts.tile([P, D], F32)
            b0b = consts.tile([P, D], F32)
            g1b = consts.tile([P, D], F32)
            b1b = consts.tile([P, D], F32)
            if not trivial:
                nc.gpsimd.dma_start(
                    bvb, bcast_ap(bv).rearrange("p (h d) -> p h d", h=H))
                for t, v in ((bob, bo), (g0b, g0), (b0b, b0),
                             (g1b, g1), (b1b, b1)):
                    nc.gpsimd.dma_start(t, bcast_ap(v))

            # ---- persistent activations ----
            kT = kT_p.tile([P, NKC, S], F32R)          # (K Wk + bk)^T
            # V rows (fp8) + ones col at DH + zero pad at DH+1 so the
            # DoubleRow Ko stride (H*(DH+2) = 528 B) is 16-byte aligned
            v_ext = vx_p.tile([P, S // P, H, DH + 2], FP8)
            qT = qT_p.tile([P, NKC, RPC], F32R)        # (Q Wq + bq)^T
            nc.vector.memset(v_ext[:, :, :, DH:DH + 1], 1.0)
            nc.vector.memset(v_ext[:, :, :, DH + 1:DH + 2], 0.0)

            def psum_drain(dst, pp, bias_ap):
                """psum -> SBUF with optional per-partition bias add"""
                if trivial:
                    nc.vector.tensor_copy(dst, pp)
                else:
                    nc.vector.tensor_scalar_add(dst, pp, bias_ap)

            # ---- phase B: qT = Wq^T Q^T + bq (Q^T DMA'd directly) ----
            def b_step(rc2, pre=None):
                if pre is None:
                    QTc = ktc_p.tile([P, NKC, SC2], F32R, tag="ktc", name="QTc")
                    nc.sync.dma_start(QTc, qt_d[:, :, rc2 * SC2:(rc2 + 1) * SC2])
                else:
                    QTc = pre
                for ci in range(NKC):
                    pp = ps_sc.tile([P, SC2], F32, tag="psc", name="ppq")
                    for kc in range(NKC):
                        nc.tensor.matmul(
                            pp,
                            lhsT=wq_t[:, kc, ci * P:(ci + 1) * P],
                            rhs=QTc[:, kc],
                            start=(kc == 0), stop=(kc == NKC - 1),
                        )
                    psum_drain(qT[:, ci, rc2 * SC2:(rc2 + 1) * SC2], pp,
                               bias_sb[:, 0, ci:ci + 1])

            b_step(0, pre=QTc0)

            # ---- helpers ----
            def a_step(sc2, pre=None):
                """project K cols [sc2*512, (sc2+1)*512) into kT and v_ext"""
                if pre is None:
                    KTc = ktc_p.tile([P, NKC, SC2], F32R, tag="ktc", name="KTc")
                    nc.sync.dma_start(KTc, kt_d[:, :, sc2 * SC2:(sc2 + 1) * SC2])
                else:
                    KTc = pre
                for ci in range(NKC):
                    pp = ps_sc.tile([P, SC2], F32, tag="psc", name="ppk")
                    for kc in range(NKC):
                        nc.tensor.matmul(
                            pp,
                            lhsT=wk_t[:, kc, ci * P:(ci + 1) * P],
                            rhs=KTc[:, kc],
                            start=(kc == 0), stop=(kc == NKC - 1),
                        )
                    psum_drain(kT[:, ci, sc2 * SC2:(sc2 + 1) * SC2], pp,
                               bias_sb[:, 1, ci:ci + 1])
                for half in range(SC2 // P):
                    pv = ps_sc.tile([P, D], F32, tag="psc", name="ppv")
                    for kc in range(NKC):
                        nc.tensor.matmul(
                            pv,
                            lhsT=KTc[:, kc, half * P:(half + 1) * P],
                            rhs=wv_t[:, kc],
                            start=(kc == 0), stop=(kc == NKC - 1),
                        )
                    sidx = sc2 * (SC2 // P) + half
                    pvh = pv.rearrange("p (h d) -> p h d", h=H)
                    if trivial:
                        nc.vector.tensor_copy(v_ext[:, sidx, :, 0:DH], pvh)
                    else:
                        nc.vector.scalar_tensor_tensor(
                            out=v_ext[:, sidx, :, 0:DH],
                            in0=pvh, scalar=1.0, in1=bvb,
                            op0=ALU.mult, op1=ALU.add,
                        )

            NSP = S // P // 2  # 16 key scpairs (256 keys each)
            PV_LAG = 5         # PV matmuls trail scores/exp by this many blocks

            def c_scores(rb, pair, i, scpair):
                """scores+exp for head 2*pair+i over keys
                [scpair*256, scpair*256+256) x queries [rb*RB, rb*RB+RB).
                Returns the fp8 exp tile for the trailing PV matmul."""
                h = 2 * pair + i
                ci, po = h // 2, (h % 2) * DH
                ps = ps_sc.tile([P, 2, RB], F32, tag="psc", name="psc")
                for j in (0, 1):
                    sc = 2 * scpair + j
                    nc.tensor.matmul(
                        ps[:, j],
                        lhsT=kT[po:po + DH, ci, sc * P:(sc + 1) * P],
                        rhs=qT[po:po + DH, ci, rb * RB:(rb + 1) * RB],
                        start=True, stop=True,
                    )
                at = at_p.tile([P, 2, RB], FP8, tag="at", name="at")
                if (scpair * 2 + i) % DVE_MOD == DVE_MOD - 1:
                    nc.vector.tensor_scalar(
                        at.bitcast(U8), ps, A_EXP, B_EXP, ALU.mult, ALU.add
                    )
                else:
                    nc.scalar.activation(at, ps, AF.Exp, scale=SCALE)
                return (at, i, scpair, h)

            def c_pv(blk):
                at, i, scpair, h, obig, post = blk
                nc.tensor.matmul(
                    obig[0:DH + 1, i],
                    lhsT=v_ext[:, 2 * scpair:2 * scpair + 2, h, 0:DH + 1],
                    rhs=at,
                    start=(scpair == 0), stop=(scpair == NSP - 1),
                    perf_mode=mybir.MatmulPerfMode.DoubleRow,
                )
                if post is not None:
                    post()

            deferred_post = []

            def c_post(rb, pair, obig, opre):
                # copy obig out of PSUM now (frees the accumulator slot);
                # defer the transpose+normalize (which needs psum slots) so
                # it doesn't contend with the next pair's score tiles
                for i, h in enumerate((2 * pair, 2 * pair + 1)):
                    ot = ot_p.tile([DH + 1, RB], F32, tag="ot", name="ot")
                    nc.vector.tensor_copy(ot, obig[0:DH + 1, i])
                    deferred_post.append((ot, h, opre))

            def c_post_finish():
                while deferred_post:
                    ot, h, opre = deferred_post.pop(0)
                    for rc in range(RB // P):
                        pt = ps_sc.tile([P, P], F32, tag="psc", name="pto")
                        nc.tensor.transpose(
                            pt[:, 0:DH + 1], ot[:, rc * P:(rc + 1) * P],
                            ident[0:DH + 1, 0:DH + 1]
                        )
                        rec = stat_p.tile([P, 1], F32, tag="rec", name="rec")
                        nc.vector.reciprocal(rec, pt[:, DH:DH + 1])
                        nc.vector.tensor_scalar_mul(
                            opre[rc][:, h], pt[:, 0:DH], rec
                        )

            def layernorm(dst, src, gb, bb, on_act=False):
                st6 = stat_p.tile([P, 6], F32, tag="st6", name="st6")
                nc.vector.bn_stats(st6, src)
                mv = stat_p.tile([P, 2], F32, tag="mv", name="mv")
                nc.vector.bn_aggr(mv, st6)
                # rstd = rsqrt(var+eps) on DVE: quake seed + 1 Newton step
                # (keeps ACT's table on Exp during the attention stream)
                srt = stat_p.tile([P, 1], F32, tag="srt", name="srt")
                nc.vector.tensor_scalar_add(srt, mv[:, 1:2], EPS)
                yv = stat_p.tile([P, 1], F32, tag="rstd", name="yv")
                yu = yv.bitcast(I32)
                nc.vector.tensor_scalar(yu, srt.bitcast(I32), 1, None,
                                        ALU.logical_shift_right)
                nc.vector.tensor_scalar(yu, yu, -1, None, ALU.bitwise_xor)
                nc.vector.tensor_scalar(yu, yu, 0x5F3759E0, None, ALU.add)
                rstd = yv
                for _it in range(1):
                    aa = stat_p.tile([P, 1], F32, tag="nsa", name="nsa")
                    nc.vector.tensor_mul(aa, rstd, rstd)
                    nc.vector.tensor_mul(aa, aa, srt)
                    nc.vector.tensor_scalar_add(aa, aa, -3.0)
                    nc.vector.scalar_tensor_tensor(
                        out=rstd, in0=rstd, scalar=-0.5, in1=aa,
                        op0=ALU.mult, op1=ALU.mult,
                    )
                if trivial:
                    if on_act:
                        # (x - mu) * rstd on ACT: x*rstd + (-mu*rstd)
                        nm = stat_p.tile([P, 1], F32, tag="nsa", name="nm")
                        nc.vector.scalar_tensor_tensor(
                            out=nm, in0=mv[:, 0:1], scalar=-1.0, in1=rstd,
                            op0=ALU.mult, op1=ALU.mult,
                        )
                        nc.scalar.activation(dst, src, AF.Identity,
                                             bias=nm, scale=rstd)
                    else:
                        nc.vector.tensor_scalar(
                            dst, src, mv[:, 0:1], rstd,
                            ALU.subtract, ALU.mult,
                        )
                    return
                xc = tail_p.tile([P, D], F32, tag="xc", name="xc")
                nc.vector.tensor_scalar_sub(xc, src, mv[:, 0:1])
                nc.vector.scalar_tensor_tensor(
                    out=dst, in0=xc, scalar=rstd, in1=gb,
                    op0=ALU.mult, op1=ALU.mult,
                )
                nc.vector.tensor_add(dst, dst, bb)

            def tail_res(rb, rc, opre, tag="ln0", on_act=False):
                """residual add + first layernorm; returns the ln0 tile"""
                gr = rb * (RB // P) + rc
                x = opre[rc].rearrange("p h d -> p (h d)")
                for kc in range(NKC):
                    pt = ps_sc.tile([P, P], F32, tag="psc", name="ptr")
                    nc.tensor.transpose(
                        pt, qT.bitcast(F32)[:, kc, gr * P:(gr + 1) * P], ident
                    )
                    nc.vector.tensor_add(
                        x[:, kc * P:(kc + 1) * P],
                        x[:, kc * P:(kc + 1) * P], pt
                    )
                ln0 = tail_p.tile([P, D], F32, tag=tag, name="ln0")
                layernorm(ln0, x, g0b, b0b, on_act=on_act)
                return ln0

            def tail_mlp(rb, rc, ln0, on_act=False):
                gr = rb * (RB // P) + rc
                lnT = tail_p.tile([P, NKC, P], F32R, tag="lnT", name="lnT")
                for kc in range(NKC):
                    pt = ps_sc.tile([P, P], F32, tag="psc", name="ptl")
                    nc.tensor.transpose(pt, ln0[:, kc * P:(kc + 1) * P], ident)
                    if on_act:
                        nc.scalar.activation(lnT[:, kc], pt, AF.Copy)
                    else:
                        nc.vector.tensor_copy(lnT[:, kc], pt)
                pm = ps_sc.tile([P, D], F32, tag="psc", name="pmo")
                for kc in range(NKC):
                    nc.tensor.matmul(
                        pm, lhsT=lnT[:, kc], rhs=wo_t[:, kc],
                        start=(kc == 0), stop=(kc == NKC - 1),
                    )
                mlp = tail_p.tile([P, D], F32, tag="xc", name="mlp_t")
                if trivial:
                    if on_act:
                        nc.scalar.activation(mlp, pm, AF.Relu)
                    else:
                        nc.vector.tensor_scalar_max(mlp, pm, 0.0)
                else:
                    nc.vector.scalar_tensor_tensor(
                        out=mlp, in0=pm, scalar=1.0, in1=bob,
                        op0=ALU.mult, op1=ALU.add,
                    )
                    nc.vector.tensor_scalar_max(mlp, mlp, 0.0)
                nc.vector.tensor_add(mlp, mlp, ln0)
                out_t = tail_p.tile([P, D], F32, tag="xc", name="out_t")
                layernorm(out_t, mlp, g1b, b1b, on_act=on_act)
                nc.sync.dma_start(Oo[gr * P:(gr + 1) * P, :], out_t)

            # ---- phase A interleaved with C(rb=0, pair=0) ----
            wv_t = w_p.tile([P, NKC, D], F32R, tag="w", name="wv_t")
            nc.gpsimd.dma_start(wv_t, Wv.rearrange("(c p) n -> p c n", p=P))

            opre0 = [opre_p.tile([P, H, DH], F32, tag="opre", name=f"opre0_{i}")
                     for i in range(RB // P)]
            opre1 = [opre_p.tile([P, H, DH], F32, tag="opre", name=f"opre1_{i}")
                     for i in range(RB // P)]
            pend = []  # rolling software-pipeline queue across pairs

            def c_emit(rb, pair, i, sp, obig, opre):
                post = None
                if sp == NSP - 1 and i == 1:
                    post = (lambda rb=rb, pair=pair, ob=obig, op=opre:
                            c_post(rb, pair, ob, op))
                pend.append(c_scores(rb, pair, i, sp) + (obig, post))
                if len(pend) > PV_LAG:
                    c_pv(pend.pop(0))

            LAG = 1
            SPP = SC2 // P // 2  # key scpairs produced per a_step (2)
            obig = ps_o.tile([P, 2, RB], F32, tag="ob", name="oacc0")
            if cc:
                # project the local key quarter into kT[:, :, 0:SQ] /
                # v_ext[:, 0:SQ//P] (staging), exchange, then stream the
                # gathered chunks back in while attention trails behind
                a_step(0, pre=KTc0)
                a_step(1)
                nc.sync.dma_start(
                    cck_i.rearrange("(c p) r -> p c r", p=P), kT[:, :, 0:SQ])
                nc.sync.dma_start(
                    ccv_i[:], v_ext[:, 0:SQ // P].rearrange("p s h d -> p s (h d)"))
                nc.gpsimd.collective_compute(
                    "AllGather", ALU.bypass, replica_groups=CC_GROUPS,
                    ins=[cck_i.ap().opt()], outs=[cck_o.ap().opt()],
                )
                nc.gpsimd.collective_compute(
                    "AllGather", ALU.bypass, replica_groups=CC_GROUPS,
                    ins=[ccv_i.ap().opt()], outs=[ccv_o.ap().opt()],
                )
                b_step(1)  # fills the collective-wait window
                cck_ov = cck_o.rearrange("q (c p) r -> q p c r", p=P)
                for g in range(8):
                    rk, hh = g // 2, g % 2
                    nc.sync.dma_start(
                        kT[:, :, g * 512:(g + 1) * 512],
                        cck_ov[rk, :, :, hh * 512:(hh + 1) * 512],
                    )
                    nc.sync.dma_start(
                        v_ext[:, g * 4:(g + 1) * 4].rearrange("p s h d -> p s (h d)"),
                        ccv_o[rk, :, hh * 4:(hh + 1) * 4],
                    )
                    if g >= LAG:
                        for sp in range(2 * (g - LAG), 2 * (g - LAG) + 2):
                            for i in (0, 1):
                                c_emit(0, 0, i, sp, obig, opre0)
                for sp in range(2 * (8 - LAG), NSP):
                    for i in (0, 1):
                        c_emit(0, 0, i, sp, obig, opre0)
            else:
                for sc2 in range(S // SC2):
                    a_step(sc2, pre=KTc0 if sc2 == 0 else None)
                    if sc2 >= LAG:
                        for sp in range(SPP * (sc2 - LAG), SPP * (sc2 - LAG) + SPP):
                            for i in (0, 1):
                                c_emit(0, 0, i, sp, obig, opre0)
                for sp in range(SPP * (S // SC2 - LAG), NSP):
                    for i in (0, 1):
                        c_emit(0, 0, i, sp, obig, opre0)

            # ---- remaining rb0 pairs (qT rb1-half emitted inside pair1) ----
            for pair in range(1, H // 2):
                obig = ps_o.tile([P, 2, RB], F32, tag="ob", name=f"oacc_r0_p{pair}")
                for sp in range(NSP):
                    for i in (0, 1):
                        c_emit(0, pair, i, sp, obig, opre0)
                    if sp == 2:
                        c_post_finish()
                    if (not cc) and pair == 1 and sp == 6:
                        b_step(1)

            wo_t = w_p.tile([P, NKC, D], F32R, tag="w", name="wo_t")
            nc.gpsimd.dma_start(wo_t, Wo.rearrange("(c p) n -> p c n", p=P))

            # ---- rb1 pairs with rb0 tails spread into their PE slack ----
            for pair in range(H // 2):
                obig = ps_o.tile([P, 2, RB], F32, tag="ob", name=f"oacc_r1_p{pair}")
                ln0_s = None
                for sp in range(NSP):
                    for i in (0, 1):
                        c_emit(1, pair, i, sp, obig, opre1)
                    if sp == 2:
                        c_post_finish()
                    elif sp == 5:
                        ln0_s = tail_res(0, pair, opre0)
                    elif sp == 10:
                        tail_mlp(0, pair, ln0_s)
            while pend:
                c_pv(pend.pop(0))
            c_post_finish()
            # final 4 row-chunks: interleave stages across chunks so the
            # layernorm latency chains overlap with PE work of other chunks;
            # ACT (done with exp by now) takes the normalize/relu/copies
            ln0_f = [tail_res(1, rc, opre1, tag=f"lnf{rc}", on_act=True)
                     for rc in range(RB // P)]
            for rc in range(RB // P):
                tail_mlp(1, rc, ln0_f[rc], on_act=True)

    nsplit, nnops = _split_multi_waits(nc)
    print(f"wait-split: {nsplit} instructions, {nnops} nops inserted")
    return nc


_cached = {}


def _get_nc(trivial):
    if trivial not in _cached:
        _cached[trivial] = build_nc(trivial=trivial)
    return _cached[trivial]


def _in_maps(Q, K, shared):
    maps = []
    KT_by_b = [np.ascontiguousarray(np.asarray(K[b]).T) for b in range(B)]
    for c in range(NC):
        b, roff = c // 4, (c % 4) * RPC
        maps.append(dict(
            shared,
            QT=np.ascontiguousarray(np.asarray(Q[b, roff:roff + RPC]).T),
            KTb=np.ascontiguousarray(
                KT_by_b[b][:, (c % 4) * SQ:(c % 4 + 1) * SQ]),
        ))
    return maps


def kernel(Q, K, Wq, bq, Wk, bk, Wv, bv, Wo, bo, g0, b0, g1, b1):
    from concourse.bass_utils import run_bass_kernel_spmd

    trivial = all(
        np.all(np.asarray(v) == c)
        for v, c in ((bq, 0), (bk, 0), (bv, 0), (bo, 0),
                     (g0, 1), (b0, 0), (g1, 1), (b1, 0))
    )
    nc = _get_nc(trivial)
    Q = np.ascontiguousarray(Q, dtype=np.float32)
    K = np.ascontiguousarray(K, dtype=np.float32)
    shared = {
        "Wq": np.ascontiguousarray(Wq, np.float32),
        "Wk": np.ascontiguousarray(Wk, np.float32),
        "Wv": np.ascontiguousarray(Wv, np.float32),
        "Wo": np.ascontiguousarray(Wo, np.float32),
        "bq": np.ascontiguousarray(bq, np.float32),
        "bk": np.ascontiguousarray(bk, np.float32),
        "bv": np.ascontiguousarray(bv, np.float32),
        "bo": np.ascontiguousarray(bo, np.float32),
        "g0": np.ascontiguousarray(g0, np.float32),
        "b0": np.ascontiguousarray(b0, np.float32),
        "g1": np.ascontiguousarray(g1, np.float32),
        "b1": np.ascontiguousarray(b1, np.float32),
    }
    res = run_bass_kernel_spmd(nc, _in_maps(Q, K, shared),
                               core_ids=list(range(NC)))
    out = np.empty((B, S, D), np.float32)
    for c in range(NC):
        b, roff = c // 4, (c % 4) * RPC
        out[b, roff:roff + RPC] = res.results[c]["O"]
    return out

